# revision 21
# baseline (speedup 1.0000x reference)
"""Dilated attention Trainium2 kernel (8 NeuronCores, SPMD).

Sharding: batch (2) x head-group (4 groups of 4 heads) -> 8 cores.
Per core (batch b, head-group g), tokens processed permuted by dilation
residue r (tok = 4m + r), so the r=0 group supplies K/V and phase 2
streams while later token groups still load.

Pipeline per core:
  x rows (residue group) --DMA f32--> cast bf16 --PE transpose--> xT
  qT/kT/v = W^T @ xT  (bf16 matmuls, fp32 psum)
  scores^T = kT-half @ qT-half  (row-packed K=64 matmul pairs)
  pT = exp(scores/8)  (1024-wide ACT calls, double-buffered psum)
  u_aug = [1|0|v]^T-contracted pT  -> row sums r + unnormalized ctx^T
  ctx^T = u * (1/r)  (DVE recip, gpsimd partition-broadcast, DVE mul)
  out partial = ctx @ Wo-rows  (fp32 psum -> fp16, strided DMA out)
Host sums the 4 per-group partials per batch and adds bo.
"""

import numpy as np

# ---- problem constants (hardcoded per contest rules) ----
B, S, E = 2, 4096, 1024
H, D = 16, 64
DIL = 4
SK = S // DIL          # 1024 dilated keys
NCORES = 8
GROUPS = 4             # head groups (cores per batch)
HPG = H // GROUPS      # 4 heads per core
CG = HPG * D           # 256 projected cols per core
SCALE = 1.0 / float(np.sqrt(D))

ET = E // 128          # 8 contraction tiles
M2 = CG // 128         # 2 col tiles (head pairs)
KT = SK // 128         # 8 ktok tiles
GS = S // DIL          # tokens per residue group (1024)
NT = GS // 512         # 512-wide q chunks per group (2)
XT = GS // 128         # x tiles per residue group (8)

_CACHE = {}


def _build_program():
    import concourse.mybir as mybir
    import concourse.tile as tile
    from concourse import bacc
    from concourse.masks import make_identity

    f32 = mybir.dt.float32
    bf16 = mybir.dt.bfloat16
    fp16 = mybir.dt.float16
    EXP = mybir.ActivationFunctionType.Exp

    nc = bacc.Bacc(None, target_bir_lowering=False)

    x_d = nc.dram_tensor("x", [S, E], f32, kind="ExternalInput")
    wq_d = nc.dram_tensor("wq", [E, CG], f32, kind="ExternalInput")
    wk_d = nc.dram_tensor("wk", [E, CG], f32, kind="ExternalInput")
    wv_d = nc.dram_tensor("wv", [E, CG], f32, kind="ExternalInput")
    wo_d = nc.dram_tensor("wo", [CG, E], f32, kind="ExternalInput")
    out_d = nc.dram_tensor("out", [S, E], fp16, kind="ExternalOutput")

    # token-permuted views: [residue, m, e]
    x_r = x_d[:].rearrange("(m f) e -> f m e", f=DIL)
    out_r = out_d[:].rearrange("(m f) e -> f m e", f=DIL)

    with tile.TileContext(nc) as tc:
        with tc.tile_pool(name="constp", bufs=1) as constp, \
             tc.tile_pool(name="wbp", bufs=1) as wbp, \
             tc.tile_pool(name="wsp", bufs=1) as wsp, \
             tc.tile_pool(name="kTp", bufs=1) as kTp, \
             tc.tile_pool(name="vp", bufs=1) as vp, \
             tc.tile_pool(name="xsp", bufs=5) as xsp, \
             tc.tile_pool(name="xbp", bufs=4) as xbp, \
             tc.tile_pool(name="xTp", bufs=3) as xTp, \
             tc.tile_pool(name="qTp", bufs=2) as qTp, \
             tc.tile_pool(name="pTp", bufs=2) as pTp, \
             tc.tile_pool(name="ctxp", bufs=2) as ctxp, \
             tc.tile_pool(name="rcpp", bufs=2) as rcpp, \
             tc.tile_pool(name="bcp", bufs=2) as bcp, \
             tc.tile_pool(name="osbp", bufs=3) as osbp, \
             tc.tile_pool(name="spp", bufs=2, space="PSUM") as spp, \
             tc.tile_pool(name="upp", bufs=2, space="PSUM") as upp, \
             tc.tile_pool(name="tpp", bufs=2, space="PSUM") as tpp:

            # ---- warm the exp activation table early ----
            dummy = rcpp.tile([1, 32], f32, name="dummy")
            nc.any.memset(dummy, 0.0)
            nc.scalar.activation(dummy, dummy, EXP)

            # identity for PE transposes
            identf = constp.tile([128, 128], f32)
            make_identity(nc, identf)
            ident = constp.tile([128, 128], bf16)
            nc.vector.tensor_copy(ident, identf)

            # warm the PE clock (HAM) with dummy matmuls during DMA wait
            wmp = upp.tile([128, 128], f32, tag="up", name="warm")
            for _ in range(40):
                nc.tensor.matmul(wmp[0:128, 0:128], lhsT=ident, rhs=ident,
                                 start=True, stop=True)

            # ---- weights on the scalar HWDGE queue (parallel with x) ----
            def loadw(dram, shape_rearr, sb_shape):
                ws = wsp.tile(sb_shape, f32, tag="ws", name=f"ws_{dram.name}")
                nc.scalar.dma_start(ws, dram[:].rearrange(shape_rearr, p=128))
                wb = wbp.tile(sb_shape, bf16, name=f"wb_{dram.name}")
                nc.vector.tensor_copy(wb, ws)
                return wb

            wk_sb = loadw(wk_d, "(k p) c -> p k c", [128, ET, CG])
            wv_sb = loadw(wv_d, "(k p) c -> p k c", [128, ET, CG])
            wq_sb = loadw(wq_d, "(k p) c -> p k c", [128, ET, CG])
            wo_sb = loadw(wo_d, "(m p) e -> p m e", [128, M2, E])

            kT = kTp.tile([128, M2, SK], bf16)
            vaug = vp.tile([128, KT, HPG, 128], bf16)
            nc.vector.memset(vaug[:, :, :, :], 0.0)
            nc.vector.memset(vaug[:, :, :, 0:1], 1.0)

            # ---- x load / cast / PE transpose for one residue group ----
            def load_group(r):
                xT = xTp.tile([128, ET, GS], bf16, tag="xT", name=f"xT{r}")
                for t in range(XT):
                    xs = xsp.tile([128, E], f32, tag="xs")
                    nc.sync.dma_start(xs, x_r[r, t * 128:(t + 1) * 128, :])
                    xsb = xbp.tile([128, E], bf16, tag="xsb")
                    if r == 0:
                        nc.scalar.copy(xsb, xs)
                    elif t % 4 == 0:
                        nc.gpsimd.tensor_copy(xsb, xs)
                    else:
                        nc.vector.tensor_copy(xsb, xs)
                    tp = tpp.tile([128, ET, 128], bf16, tag="tp")
                    for c in range(ET):
                        nc.tensor.transpose(
                            tp[:, c, :], xsb[:, c * 128:(c + 1) * 128], ident)
                    nc.vector.tensor_copy(xT[:, :, t * 128:(t + 1) * 128], tp)
                return xT

            def vproj_mt(xT0, mt):
                vp2 = upp.tile([128, 512], f32, tag="up")
                for k in range(ET):
                    nc.tensor.matmul(
                        vp2[:, 0:CG],
                        lhsT=xT0[:, k, mt * 128:(mt + 1) * 128],
                        rhs=wv_sb[:, k, :],
                        start=(k == 0), stop=(k == ET - 1))
                nc.vector.tensor_copy(
                    vaug[:, mt, :, 64:64 + D],
                    vp2[:, 0:CG].rearrange("p (h d) -> p h d", d=D))

            def kproj_nt(xT0, nt):
                for m in range(M2):
                    kp = upp.tile([128, 512], f32, tag="up")
                    for k in range(ET):
                        nc.tensor.matmul(
                            kp,
                            lhsT=wk_sb[:, k, m * 128:(m + 1) * 128],
                            rhs=xT0[:, k, nt * 512:(nt + 1) * 512],
                            start=(k == 0), stop=(k == ET - 1))
                    nc.vector.tensor_copy(
                        kT[:, m, nt * 512:(nt + 1) * 512], kp)

            def qproj_nt(qT, xT_r, nt):
                for m in range(M2):
                    qp = upp.tile([128, 512], f32, tag="up")
                    for k in range(ET):
                        nc.tensor.matmul(
                            qp,
                            lhsT=wq_sb[:, k, m * 128:(m + 1) * 128],
                            rhs=xT_r[:, k, nt * 512:(nt + 1) * 512],
                            start=(k == 0), stop=(k == ET - 1))
                    nc.vector.tensor_copy(
                        qT[:, m, nt * 512:(nt + 1) * 512], qp)

            def qproj(r, xT_r):
                qT = qTp.tile([128, M2, GS], bf16, tag="qT", name=f"qT{r}")
                for nt in range(NT):
                    qproj_nt(qT, xT_r, nt)
                return qT

            def attention(r, qT, ctxT):
                for pair in range(M2):
                    pT = pTp.tile([128, KT, 2, GS], bf16, tag="pT")
                    for mt in range(KT):
                        ks = kT[:, pair, mt * 128:(mt + 1) * 128]
                        for nt in range(NT):
                            qs = qT[:, pair, nt * 512:(nt + 1) * 512]
                            sp = spp.tile([128, 2, 512], f32, tag="sp")
                            for hl in range(2):
                                nc.tensor.matmul(
                                    sp[:, hl, :],
                                    lhsT=ks[64 * hl:64 * hl + 64, :],
                                    rhs=qs[64 * hl:64 * hl + 64, :],
                                    start=True, stop=True,
                                    tile_position=(64 * hl, 0))
                            nc.scalar.activation(
                                pT[:, mt, :, nt * 512:(nt + 1) * 512],
                                sp, EXP, scale=SCALE)
                    for hl in range(2):
                        for nt in range(NT):
                            up = upp.tile([128, 512], f32, tag="up")
                            for mt in range(KT):
                                nc.tensor.matmul(
                                    up,
                                    lhsT=vaug[:, mt, 2 * pair + hl, :],
                                    rhs=pT[:, mt, hl, nt * 512:(nt + 1) * 512],
                                    start=(mt == 0), stop=(mt == KT - 1))
                            rcp = rcpp.tile([1, 512], f32, tag="rcp")
                            with nc.allow_low_precision(reason="softmax recip"):
                                nc.vector.reciprocal_approx_fast(rcp, up[0:1, :])
                            bc = bcp.tile([64, 512], f32, tag="bc")
                            with tc.high_priority():
                                nc.gpsimd.partition_broadcast(
                                    bc, rcp, channels=64)
                            nc.vector.tensor_mul(
                                ctxT[64 * hl:64 * hl + 64, pair,
                                     nt * 512:(nt + 1) * 512],
                                up[64:128, :], bc)

            def outproj(r, ctxT):
                for ch in range(XT):
                    for ne in range(2):
                        op = upp.tile([128, 512], f32, tag="up")
                        for m in range(M2):
                            nc.tensor.matmul(
                                op,
                                lhsT=ctxT[:, m, ch * 128:(ch + 1) * 128],
                                rhs=wo_sb[:, m, ne * 512:(ne + 1) * 512],
                                start=(m == 0), stop=(m == M2 - 1))
                        osb = osbp.tile([128, 512], fp16, tag="osb")
                        nc.vector.tensor_copy(osb, op)
                        dma_eng = nc.scalar if (r == DIL - 1 and ne == 1) \
                            else nc.sync
                        dma_eng.dma_start(
                            out_r[r, ch * 128:(ch + 1) * 128,
                                  ne * 512:(ne + 1) * 512], osb)

            # ---- main schedule ----
            # r0: fuse projections into the load, tile by tile
            xT0 = xTp.tile([128, ET, GS], bf16, tag="xT", name="xT0")
            qT_cur = qTp.tile([128, M2, GS], bf16, tag="qT", name="qT0")
            for t in range(XT):
                xs = xsp.tile([128, E], f32, tag="xs")
                nc.sync.dma_start(xs, x_r[0, t * 128:(t + 1) * 128, :])
                xsb = xbp.tile([128, E], bf16, tag="xsb")
                nc.scalar.copy(xsb, xs)
                tp = tpp.tile([128, ET, 128], bf16, tag="tp")
                for c in range(ET):
                    nc.tensor.transpose(
                        tp[:, c, :], xsb[:, c * 128:(c + 1) * 128], ident)
                nc.vector.tensor_copy(xT0[:, :, t * 128:(t + 1) * 128], tp)
                vproj_mt(xT0, t)
                if t % 4 == 3:
                    kproj_nt(xT0, t // 4)
                    qproj_nt(qT_cur, xT0, t // 4)
            xT_next = load_group(1)
            xT_next2 = load_group(2)
            for r in range(DIL):
                ctxT = ctxp.tile([128, M2, GS], bf16, tag="ctxT")
                attention(r, qT_cur, ctxT)
                if r + 1 < DIL:
                    qT_cur = qproj(r + 1, xT_next)
                    xT_next = xT_next2
                    if r + 3 < DIL:
                        xT_next2 = load_group(r + 3)
                outproj(r, ctxT)

    nc.compile()
    return nc


def _get_program():
    if "nc" not in _CACHE:
        _CACHE["nc"] = _build_program()
    return _CACHE["nc"]


def make_in_maps(x, Wq, bq, Wk, bk, Wv, bv, Wo, bo):
    in_maps = []
    for c in range(NCORES):
        b, g = c // GROUPS, c % GROUPS
        cs = slice(g * CG, (g + 1) * CG)
        in_maps.append({
            "x": np.ascontiguousarray(np.asarray(x[b], dtype=np.float32)),
            "wq": np.ascontiguousarray(np.asarray(Wq[:, cs], dtype=np.float32)),
            "wk": np.ascontiguousarray(np.asarray(Wk[:, cs], dtype=np.float32)),
            "wv": np.ascontiguousarray(np.asarray(Wv[:, cs], dtype=np.float32)),
            "wo": np.ascontiguousarray(np.asarray(Wo[cs, :], dtype=np.float32)),
        })
    return in_maps


def gather_output(results, bo):
    out = np.zeros((B, S, E), dtype=np.float32)
    for c in range(NCORES):
        b = c // GROUPS
        out[b] += results[c]["out"].astype(np.float32)
    out += np.asarray(bo, dtype=np.float32)
    return out


def kernel(x, Wq, bq, Wk, bk, Wv, bv, Wo, bo, _trace=False):
    from concourse import bass_utils

    nc = _get_program()
    in_maps = make_in_maps(x, Wq, bq, Wk, bk, Wv, bv, Wo, bo)
    res = bass_utils.run_bass_kernel_spmd(
        nc, in_maps, core_ids=list(range(NCORES)), trace=_trace)
    _CACHE["last_result"] = res
    return gather_output(res.results, bo)


# revision 22
# speedup vs baseline: 1.0144x; 1.0144x over previous
"""Dilated attention Trainium2 kernel (8 NeuronCores, SPMD).

Sharding: batch (2) x head-group (4 groups of 4 heads) -> 8 cores.
Per core (batch b, head-group g), tokens processed permuted by dilation
residue r (tok = 4m + r), so the r=0 group supplies K/V and phase 2
streams while later token groups still load.

Pipeline per core:
  x rows (residue group) --DMA f32--> cast bf16 --PE transpose--> xT
  qT/kT/v = W^T @ xT  (bf16 matmuls, fp32 psum)
  scores^T = kT-half @ qT-half  (row-packed K=64 matmul pairs)
  pT = exp(scores/8)  (1024-wide ACT calls, double-buffered psum)
  u_aug = [1|0|v]^T-contracted pT  -> row sums r + unnormalized ctx^T
  ctx^T = u * (1/r)  (DVE recip, gpsimd partition-broadcast, DVE mul)
  out partial = ctx @ Wo-rows  (fp32 psum -> fp16, strided DMA out)
Host sums the 4 per-group partials per batch and adds bo.
"""

import numpy as np

# ---- problem constants (hardcoded per contest rules) ----
B, S, E = 2, 4096, 1024
H, D = 16, 64
DIL = 4
SK = S // DIL          # 1024 dilated keys
NCORES = 8
GROUPS = 4             # head groups (cores per batch)
HPG = H // GROUPS      # 4 heads per core
CG = HPG * D           # 256 projected cols per core
SCALE = 1.0 / float(np.sqrt(D))

ET = E // 128          # 8 contraction tiles
M2 = CG // 128         # 2 col tiles (head pairs)
KT = SK // 128         # 8 ktok tiles
GS = S // DIL          # tokens per residue group (1024)
NT = GS // 512         # 512-wide q chunks per group (2)
XT = GS // 128         # x tiles per residue group (8)

_CACHE = {}


def _build_program():
    import concourse.mybir as mybir
    import concourse.tile as tile
    from concourse import bacc
    from concourse.masks import make_identity

    f32 = mybir.dt.float32
    bf16 = mybir.dt.bfloat16
    fp16 = mybir.dt.float16
    EXP = mybir.ActivationFunctionType.Exp

    nc = bacc.Bacc(None, target_bir_lowering=False)

    x_d = nc.dram_tensor("x", [S, E], f32, kind="ExternalInput")
    wq_d = nc.dram_tensor("wq", [E, CG], f32, kind="ExternalInput")
    wk_d = nc.dram_tensor("wk", [E, CG], f32, kind="ExternalInput")
    wv_d = nc.dram_tensor("wv", [E, CG], f32, kind="ExternalInput")
    wo_d = nc.dram_tensor("wo", [CG, E], f32, kind="ExternalInput")
    out_d = nc.dram_tensor("out", [S, E], fp16, kind="ExternalOutput")

    # token-permuted views: [residue, m, e]
    x_r = x_d[:].rearrange("(m f) e -> f m e", f=DIL)
    out_r = out_d[:].rearrange("(m f) e -> f m e", f=DIL)

    with tile.TileContext(nc) as tc:
        with tc.tile_pool(name="constp", bufs=1) as constp, \
             tc.tile_pool(name="wbp", bufs=1) as wbp, \
             tc.tile_pool(name="wsp", bufs=1) as wsp, \
             tc.tile_pool(name="kTp", bufs=1) as kTp, \
             tc.tile_pool(name="vp", bufs=1) as vp, \
             tc.tile_pool(name="xsp", bufs=5) as xsp, \
             tc.tile_pool(name="xbp", bufs=4) as xbp, \
             tc.tile_pool(name="xTp", bufs=3) as xTp, \
             tc.tile_pool(name="qTp", bufs=2) as qTp, \
             tc.tile_pool(name="pTp", bufs=2) as pTp, \
             tc.tile_pool(name="ctxp", bufs=2) as ctxp, \
             tc.tile_pool(name="rcpp", bufs=2) as rcpp, \
             tc.tile_pool(name="bcp", bufs=2) as bcp, \
             tc.tile_pool(name="osbp", bufs=3) as osbp, \
             tc.tile_pool(name="spp", bufs=2, space="PSUM") as spp, \
             tc.tile_pool(name="upp", bufs=2, space="PSUM") as upp, \
             tc.tile_pool(name="tpp", bufs=2, space="PSUM") as tpp:

            # ---- warm the exp activation table early ----
            dummy = rcpp.tile([1, 32], f32, name="dummy")
            nc.any.memset(dummy, 0.0)
            nc.scalar.activation(dummy, dummy, EXP)

            # identity for PE transposes
            identf = constp.tile([128, 128], f32)
            make_identity(nc, identf)
            ident = constp.tile([128, 128], bf16)
            nc.vector.tensor_copy(ident, identf)

            # ---- weights on the scalar HWDGE queue (parallel with x) ----
            def loadw(dram, shape_rearr, sb_shape):
                ws = wsp.tile(sb_shape, f32, tag="ws", name=f"ws_{dram.name}")
                nc.scalar.dma_start(ws, dram[:].rearrange(shape_rearr, p=128))
                wb = wbp.tile(sb_shape, bf16, name=f"wb_{dram.name}")
                nc.vector.tensor_copy(wb, ws)
                return wb

            wk_sb = loadw(wk_d, "(k p) c -> p k c", [128, ET, CG])
            wv_sb = loadw(wv_d, "(k p) c -> p k c", [128, ET, CG])
            wq_sb = loadw(wq_d, "(k p) c -> p k c", [128, ET, CG])
            wo_sb = loadw(wo_d, "(m p) e -> p m e", [128, M2, E])

            kT = kTp.tile([128, M2, SK], bf16)
            vaug = vp.tile([128, KT, HPG, 128], bf16)
            nc.vector.memset(vaug[:, :, :, :], 0.0)
            nc.vector.memset(vaug[:, :, :, 0:1], 1.0)

            # ---- x load / cast / PE transpose for one residue group ----
            def load_group(r):
                xT = xTp.tile([128, ET, GS], bf16, tag="xT", name=f"xT{r}")
                for t in range(XT):
                    xs = xsp.tile([128, E], f32, tag="xs")
                    nc.sync.dma_start(xs, x_r[r, t * 128:(t + 1) * 128, :])
                    xsb = xbp.tile([128, E], bf16, tag="xsb")
                    if r == 0:
                        nc.scalar.copy(xsb, xs)
                    elif t % 4 == 0:
                        nc.gpsimd.tensor_copy(xsb, xs)
                    else:
                        nc.vector.tensor_copy(xsb, xs)
                    tp = tpp.tile([128, ET, 128], bf16, tag="tp")
                    for c in range(ET):
                        nc.tensor.transpose(
                            tp[:, c, :], xsb[:, c * 128:(c + 1) * 128], ident)
                    nc.vector.tensor_copy(xT[:, :, t * 128:(t + 1) * 128], tp)
                return xT

            def vproj_mt(xT0, mt):
                vp2 = upp.tile([128, 512], f32, tag="up")
                for k in range(ET):
                    nc.tensor.matmul(
                        vp2[:, 0:CG],
                        lhsT=xT0[:, k, mt * 128:(mt + 1) * 128],
                        rhs=wv_sb[:, k, :],
                        start=(k == 0), stop=(k == ET - 1))
                nc.vector.tensor_copy(
                    vaug[:, mt, :, 64:64 + D],
                    vp2[:, 0:CG].rearrange("p (h d) -> p h d", d=D))

            def kproj_nt(xT0, nt):
                for m in range(M2):
                    kp = upp.tile([128, 512], f32, tag="up")
                    for k in range(ET):
                        nc.tensor.matmul(
                            kp,
                            lhsT=wk_sb[:, k, m * 128:(m + 1) * 128],
                            rhs=xT0[:, k, nt * 512:(nt + 1) * 512],
                            start=(k == 0), stop=(k == ET - 1))
                    nc.vector.tensor_copy(
                        kT[:, m, nt * 512:(nt + 1) * 512], kp)

            def qproj_nt(qT, xT_r, nt):
                for m in range(M2):
                    qp = upp.tile([128, 512], f32, tag="up")
                    for k in range(ET):
                        nc.tensor.matmul(
                            qp,
                            lhsT=wq_sb[:, k, m * 128:(m + 1) * 128],
                            rhs=xT_r[:, k, nt * 512:(nt + 1) * 512],
                            start=(k == 0), stop=(k == ET - 1))
                    nc.vector.tensor_copy(
                        qT[:, m, nt * 512:(nt + 1) * 512], qp)

            def qproj(r, xT_r):
                qT = qTp.tile([128, M2, GS], bf16, tag="qT", name=f"qT{r}")
                for nt in range(NT):
                    qproj_nt(qT, xT_r, nt)
                return qT

            def attention(r, qT, ctxT):
                for pair in range(M2):
                    pT = pTp.tile([128, KT, 2, GS], bf16, tag="pT")
                    for mt in range(KT):
                        ks = kT[:, pair, mt * 128:(mt + 1) * 128]
                        for nt in range(NT):
                            qs = qT[:, pair, nt * 512:(nt + 1) * 512]
                            sp = spp.tile([128, 2, 512], f32, tag="sp")
                            for hl in range(2):
                                nc.tensor.matmul(
                                    sp[:, hl, :],
                                    lhsT=ks[64 * hl:64 * hl + 64, :],
                                    rhs=qs[64 * hl:64 * hl + 64, :],
                                    start=True, stop=True,
                                    tile_position=(64 * hl, 0))
                            nc.scalar.activation(
                                pT[:, mt, :, nt * 512:(nt + 1) * 512],
                                sp, EXP, scale=SCALE)
                    for hl in range(2):
                        for nt in range(NT):
                            up = upp.tile([128, 512], f32, tag="up")
                            for mt in range(KT):
                                nc.tensor.matmul(
                                    up,
                                    lhsT=vaug[:, mt, 2 * pair + hl, :],
                                    rhs=pT[:, mt, hl, nt * 512:(nt + 1) * 512],
                                    start=(mt == 0), stop=(mt == KT - 1))
                            rcp = rcpp.tile([1, 512], f32, tag="rcp")
                            with nc.allow_low_precision(reason="softmax recip"):
                                nc.vector.reciprocal_approx_fast(rcp, up[0:1, :])
                            bc = bcp.tile([64, 512], f32, tag="bc")
                            with tc.high_priority():
                                nc.gpsimd.partition_broadcast(
                                    bc, rcp, channels=64)
                            nc.vector.tensor_mul(
                                ctxT[64 * hl:64 * hl + 64, pair,
                                     nt * 512:(nt + 1) * 512],
                                up[64:128, :], bc)

            def outproj(r, ctxT):
                for ch in range(XT):
                    for ne in range(2):
                        op = upp.tile([128, 512], f32, tag="up")
                        for m in range(M2):
                            nc.tensor.matmul(
                                op,
                                lhsT=ctxT[:, m, ch * 128:(ch + 1) * 128],
                                rhs=wo_sb[:, m, ne * 512:(ne + 1) * 512],
                                start=(m == 0), stop=(m == M2 - 1))
                        osb = osbp.tile([128, 512], fp16, tag="osb")
                        if ne == 1:
                            nc.scalar.copy(osb, op)
                        else:
                            nc.vector.tensor_copy(osb, op)
                        dma_eng = nc.scalar if (r == DIL - 1 and ne == 1) \
                            else nc.sync
                        dma_eng.dma_start(
                            out_r[r, ch * 128:(ch + 1) * 128,
                                  ne * 512:(ne + 1) * 512], osb)

            # ---- main schedule ----
            # r0: fuse projections into the load, tile by tile
            xT0 = xTp.tile([128, ET, GS], bf16, tag="xT", name="xT0")
            qT_cur = qTp.tile([128, M2, GS], bf16, tag="qT", name="qT0")
            for t in range(XT):
                xs = xsp.tile([128, E], f32, tag="xs")
                nc.sync.dma_start(xs, x_r[0, t * 128:(t + 1) * 128, :])
                xsb = xbp.tile([128, E], bf16, tag="xsb")
                nc.scalar.copy(xsb, xs)
                tp = tpp.tile([128, ET, 128], bf16, tag="tp")
                for c in range(ET):
                    nc.tensor.transpose(
                        tp[:, c, :], xsb[:, c * 128:(c + 1) * 128], ident)
                nc.vector.tensor_copy(xT0[:, :, t * 128:(t + 1) * 128], tp)
                vproj_mt(xT0, t)
                if t % 4 == 3:
                    kproj_nt(xT0, t // 4)
                    qproj_nt(qT_cur, xT0, t // 4)
            xT_next = load_group(1)
            xT_next2 = load_group(2)
            for r in range(DIL):
                ctxT = ctxp.tile([128, M2, GS], bf16, tag="ctxT")
                attention(r, qT_cur, ctxT)
                if r + 1 < DIL:
                    qT_cur = qproj(r + 1, xT_next)
                    xT_next = xT_next2
                    if r + 3 < DIL:
                        xT_next2 = load_group(r + 3)
                outproj(r, ctxT)

    nc.compile()
    return nc


def _get_program():
    if "nc" not in _CACHE:
        _CACHE["nc"] = _build_program()
    return _CACHE["nc"]


def make_in_maps(x, Wq, bq, Wk, bk, Wv, bv, Wo, bo):
    in_maps = []
    for c in range(NCORES):
        b, g = c // GROUPS, c % GROUPS
        cs = slice(g * CG, (g + 1) * CG)
        in_maps.append({
            "x": np.ascontiguousarray(np.asarray(x[b], dtype=np.float32)),
            "wq": np.ascontiguousarray(np.asarray(Wq[:, cs], dtype=np.float32)),
            "wk": np.ascontiguousarray(np.asarray(Wk[:, cs], dtype=np.float32)),
            "wv": np.ascontiguousarray(np.asarray(Wv[:, cs], dtype=np.float32)),
            "wo": np.ascontiguousarray(np.asarray(Wo[cs, :], dtype=np.float32)),
        })
    return in_maps


def gather_output(results, bo):
    out = np.zeros((B, S, E), dtype=np.float32)
    for c in range(NCORES):
        b = c // GROUPS
        out[b] += results[c]["out"].astype(np.float32)
    out += np.asarray(bo, dtype=np.float32)
    return out


def kernel(x, Wq, bq, Wk, bk, Wv, bv, Wo, bo, _trace=False):
    from concourse import bass_utils

    nc = _get_program()
    in_maps = make_in_maps(x, Wq, bq, Wk, bk, Wv, bv, Wo, bo)
    res = bass_utils.run_bass_kernel_spmd(
        nc, in_maps, core_ids=list(range(NCORES)), trace=_trace)
    _CACHE["last_result"] = res
    return gather_output(res.results, bo)
